# revision 2
# baseline (speedup 1.0000x reference)
"""FFM layer kernel for 8 Trainium2 NeuronCores — fp8 DoubleRow version.

Math (reference): x[B,39] = 13 dense cols + 26 sparse index cols (ints 0..99
stored as f32).  inputs[B,2613] = [dense | one_hot(sparse)], then
  linear = inputs @ w.T + b
  field  = einsum('bn,nfk->bfk', inputs, v)        # [B,39,16]
  cross  = 0.5*sum_k((sum_f field)^2 - sum_f field^2)
  out    = sigmoid(linear + cross)

Strategy: data-parallel over batch, 2048 rows/core.  On each core the one-hot
matrix is built on-device in fp8e4 (is_equal against an offset ramp, split
across the DVE and GpSimd engines), then used as the stationary operand of
fp8 DoubleRow matmuls (2 contraction chunks of 128 per pass, 2x PE
throughput):
  psum[128b, 658] = sum_pairs ohpair[128f,2,128b].T @ vp_pair[128f,2,658]
Columns: 0..623 = field (f*16+k) scaled 16x, 624..639 = V1hi = e4m3(sum_f v),
640..655 = V1res = e4m3(16*(V1-V1hi)), 656 = w_hi, 657 = w_res (16x).  The
dedicated V1/w columns give a near-exact s[b,k] and linear part so the fp8
quantization error of v only enters through sum_f field^2 (rel err ~5e-3,
budget 2e-2).  Feature rows: [1s row (bias via w cols) | 13 x_hi | 13 x_res
| pad to 32 | 26*100 one-hot | zero tail] = 21 chunks of 128; 10 DoubleRow
pairs + 1 single fp8 matmul (72 live rows).  fp32 PSUM.

Epilogue per batch tile: s = V1hi_col + V1res_col/16 (DVE), sq via scalar
Square+accumulate, sigmoid on scalar.  Epilogue vector ops for group g are
deferred until after the one-hot builds of group g+1 are issued so the DVE
queue never blocks chunk production; scalar Square runs inline.  Throwaway
fp8 warmup matmuls release the HAM clock throttle during the DMA head.
"""

import sys

sys.path.insert(0, "/opt/trn_rl_repo")

import numpy as np
import ml_dtypes

import concourse.tile as tile
from concourse import bacc, mybir
from concourse.bass_utils import run_bass_kernel_spmd

N_CORES = 8
B_FULL = 16384
BC = B_FULL // N_CORES  # 2048 rows per core
P = 128
N_DENSE = 13
N_SPARSE = 26
SPARSE_DIM = 100
N_FIELD = 39
K_DIM = 16
NCHUNK = 21
NPAIR = 10              # chunks 0..19 as DoubleRow pairs; chunk 20 single
LAST_K = 72             # live rows in chunk 20 (2632 - 2560)
RTOT = NCHUNK * P       # 2688 padded feature rows
SP0 = 32                # first one-hot row
NFEAT_END = SP0 + N_SPARSE * SPARSE_DIM  # 2632
NFK = N_FIELD * K_DIM   # 624
COLS = NFK + 2 * K_DIM + 2  # 658
CSPLIT = 512
VSCALE = 16.0           # field cols store 16*v; sq_raw = 256*sq
GB = 4                  # batch tiles per group (4 psum tiles = 8 banks)
GSPLIT = 12             # chunks 0..11 built on DVE, 12..20 on GpSimd

F8 = mybir.dt.float8e4
F32 = mybir.dt.float32
I8 = mybir.dt.int8
E4NP = ml_dtypes.float8_e4m3

_prog_cache = {}


def _q8(a):
    """Round-trip through TRN e4m3 (numpy f32 in/out)."""
    return np.clip(a, -240, 240).astype(E4NP).astype(np.float32)


def _build_program(bc):
    """One SPMD program for a batch slice of `bc` rows (all cores identical)."""
    nbt = bc // P
    ngroups = nbt // GB
    assert nbt % GB == 0
    gw = GB * P  # one-hot column width built per group

    nc = bacc.Bacc("TRN2", target_bir_lowering=False, debug=False)
    idx_d = nc.declare_dram_parameter(
        "idxrep", [ngroups, P, NCHUNK, gw], I8, isOutput=False)
    xdn_d = nc.declare_dram_parameter("xdn", [SP0, bc], F8, isOutput=False)
    vp_d = nc.declare_dram_parameter(
        "vperm", [P, NCHUNK, COLS], F8, isOutput=False)
    ramp_d = nc.declare_dram_parameter("ramp", [P, NCHUNK], F32, isOutput=False)
    y_d = nc.declare_dram_parameter("y", [P, nbt], F32, isOutput=True)

    ISUB = [(0, 2), (2, 7), (7, 11), (11, 16), (16, NCHUNK)]
    ISUB_ENG = ("sync", "scalar", "sync", "scalar", "sync")
    VSUB = [(0, 2), (2, 9), (9, NCHUNK)]
    VSUB_ENG = ("scalar", "sync", "scalar")

    DR = mybir.MatmulPerfMode.DoubleRow

    with tile.TileContext(nc) as tc:
        with (
            tc.tile_pool(name="pers", bufs=1) as pers,
            tc.tile_pool(name="idxp", bufs=3) as idxp,
            tc.tile_pool(name="psum", bufs=4, space="PSUM") as psum,
            tc.tile_pool(name="epi", bufs=3) as epi,
        ):
            # one-hot stationary: 10 pair tiles + 1 single-chunk tile
            oh_p = [pers.tile([P, 2, bc], F8, tag=f"ohp{j}", name=f"ohp{j}")
                    for j in range(NPAIR)]
            oh_l = pers.tile([P, 1, bc], F8, tag="ohl", name="ohl")

            def oh_slice(c, r0, r1, j0, j1):
                if c < 2 * NPAIR:
                    return oh_p[c // 2][r0:r1, c % 2, j0:j1]
                return oh_l[r0:r1, 0, j0:j1]

            y_all = pers.tile([P, nbt], F32, tag="yall")
            vp_all = pers.tile([P, NCHUNK, COLS], F8, tag="vp")

            def load_idx(g):
                c0 = g * gw
                subs = []
                for (lo, hi), ename in zip(ISUB, ISUB_ENG):
                    eng = getattr(nc, ename)
                    it = idxp.tile([P, hi - lo, gw], I8, tag=f"idx{lo}",
                                   name="idx", bufs=3)
                    eng.dma_start(it[:], idx_d[g, :, lo:hi, :])
                    subs.append((lo, it))
                return subs

            ramp_t = pers.tile([P, NCHUNK], F32, tag="ramp")
            nc.sync.dma_start(ramp_t[:], ramp_d[:])

            def load_vp(lo, hi, ename):
                getattr(nc, ename).dma_start(
                    vp_all[:, lo:hi, :], vp_d[:, lo:hi, :])

            load_vp(*VSUB[0], VSUB_ENG[0])
            xdn_t = pers.tile([SP0, bc], F8, tag="xdn")
            nc.scalar.dma_start(xdn_t[:], xdn_d[:])
            idx_subs = {0: load_idx(0)}
            for (lo, hi), ename in list(zip(VSUB, VSUB_ENG))[1:]:
                load_vp(lo, hi, ename)
            idx_subs[1] = load_idx(1)

            # PE warmup: throwaway fp8 DoubleRow matmuls on zeroed tiles
            # during the DMA head release the HAM clock throttle so the real
            # matmuls start at speed
            wz16 = pers.tile([P, 2, 16], F8, tag="wz16")
            wz512 = pers.tile([P, 2, 512], F8, tag="wz512")
            nc.gpsimd.memset(wz16[:], 0.0)
            nc.gpsimd.memset(wz512[:], 0.0)
            wps = psum.tile([P, COLS], F32, tag="ps", name="warmps")
            for _ in range(10):
                nc.tensor.matmul(wps[0:16, 0:512], wz16[:], wz512[:],
                                 start=True, stop=True, perf_mode=DR)
            for _ in range(40):
                nc.tensor.matmul(wps[0:16, 0:64], wz16[:], wz512[:, :, 0:64],
                                 start=True, stop=True, perf_mode=DR)

            def build_group(g, subs):
                c0 = g * gw
                passes = ((0, P), (P, gw)) if g == 0 else ((0, gw),)
                for pj0, pj1 in passes:
                    for si, (lo, it) in enumerate(subs):
                        for ci in range(it.shape[1]):
                            c = lo + ci
                            eng = nc.vector if c < GSPLIT else nc.gpsimd
                            rngs = ((SP0, 64), (64, P)) if c == 0 else ((0, P),)
                            for r0, r1 in rngs:
                                eng.tensor_scalar(
                                    out=oh_slice(c, r0, r1, c0 + pj0, c0 + pj1),
                                    in0=it[r0:r1, ci, pj0:pj1],
                                    scalar1=ramp_t[r0:r1, c:c + 1],
                                    scalar2=None,
                                    op0=mybir.AluOpType.is_equal,
                                )
                        if si == 0:
                            nc.vector.tensor_copy(
                                oh_slice(0, 0, SP0, c0 + pj0, c0 + pj1),
                                xdn_t[:, c0 + pj0:c0 + pj1])

            def issue_matmuls(bt):
                ps = psum.tile([P, COLS], F32, tag="ps")
                b0, b1 = bt * P, (bt + 1) * P
                for j in range(NPAIR + 1):
                    if j < NPAIR:
                        lhs = oh_p[j][:, :, b0:b1]
                        r_lo = vp_all[:, 2 * j:2 * j + 2, 0:CSPLIT]
                        r_hi = vp_all[:, 2 * j:2 * j + 2, CSPLIT:COLS]
                        pm = DR
                    else:
                        lhs = oh_l[0:LAST_K, 0, b0:b1]
                        r_lo = vp_all[0:LAST_K, 2 * NPAIR, 0:CSPLIT]
                        r_hi = vp_all[0:LAST_K, 2 * NPAIR, CSPLIT:COLS]
                        pm = None
                    st = (j == 0)
                    sp = (j == NPAIR)
                    nc.tensor.matmul(ps[:, 0:CSPLIT], lhs, r_lo,
                                     start=st, stop=sp, perf_mode=pm)
                    nc.tensor.matmul(ps[:, CSPLIT:COLS], lhs, r_hi,
                                     start=st, stop=sp, perf_mode=pm)
                return ps

            def issue_sq(bt, ps):
                """Scalar-engine Square+accumulate over the field columns."""
                sq_scr = epi.tile([P, NFK], F32, tag="sqscr")
                sqsum = epi.tile([P, 1], F32, tag="sqsum")
                nc.scalar.activation(
                    out=sq_scr[:], in_=ps[:, 0:NFK],
                    func=mybir.ActivationFunctionType.Square,
                    accum_out=sqsum[:],
                )
                return sqsum

            def issue_epi(bt, ps, sqsum):
                """DVE combine ops + scalar s2/sigmoid for one batch tile."""
                sres = epi.tile([P, K_DIM], F32, tag="sres")
                nc.vector.tensor_scalar(
                    out=sres[:], in0=ps[:, NFK + K_DIM:NFK + 2 * K_DIM],
                    scalar1=1.0 / VSCALE, scalar2=None,
                    op0=mybir.AluOpType.mult,
                )
                s_t = epi.tile([P, K_DIM], F32, tag="s")
                nc.vector.tensor_tensor(
                    out=s_t[:], in0=sres[:], in1=ps[:, NFK:NFK + K_DIM],
                    op=mybir.AluOpType.add,
                )
                lin = epi.tile([P, 1], F32, tag="lin")
                nc.vector.tensor_scalar(
                    out=lin[:], in0=ps[:, COLS - 1:COLS],
                    scalar1=1.0 / VSCALE, scalar2=ps[:, COLS - 2:COLS - 1],
                    op0=mybir.AluOpType.mult,
                    op1=mybir.AluOpType.add,
                )
                b2 = epi.tile([P, 1], F32, tag="b2")
                nc.vector.tensor_scalar(
                    out=b2[:], in0=sqsum[:],
                    scalar1=-0.5 / (VSCALE * VSCALE), scalar2=lin[:],
                    op0=mybir.AluOpType.mult,
                    op1=mybir.AluOpType.add,
                )
                s2_scr = epi.tile([P, K_DIM], F32, tag="s2scr")
                s2sum = epi.tile([P, 1], F32, tag="s2sum")
                nc.scalar.activation(
                    out=s2_scr[:], in_=s_t[:],
                    func=mybir.ActivationFunctionType.Square,
                    accum_out=s2sum[:],
                )
                nc.scalar.activation(
                    out=y_all[:, bt:bt + 1], in_=s2sum[:],
                    func=mybir.ActivationFunctionType.Sigmoid,
                    scale=0.5, bias=b2[:],
                )

            pending = []
            for g in range(ngroups):
                if g + 2 < ngroups:
                    idx_subs[g + 2] = load_idx(g + 2)
                build_group(g, idx_subs.pop(g))
                for bt, ps, sqsum in pending:
                    issue_epi(bt, ps, sqsum)
                pending = []
                last = (g == ngroups - 1)
                for b4 in range(GB):
                    bt = g * GB + b4
                    ps = issue_matmuls(bt)
                    sqsum = issue_sq(bt, ps)
                    if last:
                        issue_epi(bt, ps, sqsum)
                    else:
                        pending.append((bt, ps, sqsum))
            nc.sync.dma_start(y_d[:], y_all[:])

    nc.compile()
    return nc


def _get_program(bc):
    if bc not in _prog_cache:
        _prog_cache[bc] = _build_program(bc)
    return _prog_cache[bc]


def _prep_shared(w_weight, w_bias, v):
    """vperm[128, 21, 658] e4m3 and ramp[128, 21] f32 (same on every core)."""
    v2 = v.reshape(2613, NFK)            # col = f*16 + k
    V1 = v.sum(axis=1)                   # [2613, 16]
    V1hi = _q8(V1)
    V1res = _q8(VSCALE * (V1 - V1hi))
    w = w_weight[0]
    whi = _q8(w)
    wres = _q8(VSCALE * (w - whi))

    vp = np.zeros((RTOT, COLS), np.float32)

    def fill(rows, n0, n1):
        vp[rows, 0:NFK] = _q8(VSCALE * v2[n0:n1])
        vp[rows, NFK:NFK + K_DIM] = V1hi[n0:n1]
        vp[rows, NFK + K_DIM:NFK + 2 * K_DIM] = V1res[n0:n1]
        vp[rows, COLS - 2] = whi[n0:n1]
        vp[rows, COLS - 1] = wres[n0:n1]

    fill(slice(1, 1 + N_DENSE), 0, N_DENSE)          # x_hi rows
    fill(slice(14, 14 + N_DENSE), 0, N_DENSE)        # x_res rows
    fill(slice(SP0, NFEAT_END), N_DENSE, 2613)       # one-hot rows
    b = float(w_bias[0])
    bhi = _q8(np.float32(b))
    vp[0, COLS - 2] = bhi
    vp[0, COLS - 1] = _q8(np.float32(VSCALE * (b - bhi)))
    vp8 = np.ascontiguousarray(
        vp.astype(E4NP).reshape(NCHUNK, P, COLS).transpose(1, 0, 2))

    r = np.arange(RTOT)
    in_sparse = (r >= SP0) & (r < NFEAT_END)
    off = np.where(in_sparse, (r - SP0) % SPARSE_DIM, 0)
    ramp = np.ascontiguousarray(off.reshape(NCHUNK, P).T.astype(np.float32))
    s_of_r = np.where(in_sparse, (r - SP0) // SPARSE_DIM, -1)
    return vp8, ramp, s_of_r, in_sparse


def _prep_core(x_core, s_of_r, in_sparse):
    """Per-core idxrep[RTOT, bc] int8 and dense xdn[32, bc] e4m3."""
    bc = x_core.shape[0]
    idxrep = np.full((RTOT, bc), -1, np.int8)
    cols = (N_DENSE + s_of_r[in_sparse]).astype(np.int64)
    idxrep[in_sparse] = x_core[:, cols].T.astype(np.int8)
    ngroups = bc // (GB * P)
    gw = GB * P
    idxrep = np.ascontiguousarray(
        idxrep.reshape(NCHUNK, P, ngroups, gw).transpose(2, 1, 0, 3))
    xd = x_core[:, :N_DENSE].T.astype(np.float32)    # [13, bc]
    xhi = _q8(xd)
    xres = _q8(xd - xhi)
    xdn = np.zeros((SP0, bc), np.float32)
    xdn[0] = 1.0
    xdn[1:1 + N_DENSE] = xhi
    xdn[14:14 + N_DENSE] = xres
    return idxrep, xdn.astype(E4NP)


def run(x, w_weight, w_bias, v, trace=False, trace_kwargs=None):
    x = np.asarray(x, np.float32)
    w_weight = np.asarray(w_weight, np.float32)
    w_bias = np.asarray(w_bias, np.float32)
    v = np.asarray(v, np.float32)
    assert x.shape == (B_FULL, 39), x.shape

    vp8, ramp, s_of_r, in_sparse = _prep_shared(w_weight, w_bias, v)
    in_maps = []
    for i in range(N_CORES):
        xc = x[i * BC:(i + 1) * BC]
        idxrep, xdn = _prep_core(xc, s_of_r, in_sparse)
        in_maps.append({
            "idxrep": idxrep,
            "xdn": xdn,
            "vperm": vp8,
            "ramp": ramp,
        })

    nc = _get_program(BC)
    res = run_bass_kernel_spmd(
        nc, in_maps, list(range(N_CORES)),
        trace=trace, **(trace_kwargs or {}),
    )
    y = np.concatenate(
        [res.results[i]["y"].T.reshape(-1, 1) for i in range(N_CORES)], axis=0
    )
    return y.astype(np.float32), res


def kernel(x, w_weight, w_bias, v):
    y, _ = run(x, w_weight, w_bias, v)
    return y
